# revision 11
# baseline (speedup 1.0000x reference)
"""Trainium2 Bass kernel for nn_MemoryModule (scatter_memory).

Computes, for z (B,H) and memory_items (N,H):
  read path : z_hat = softmax(cos_sim(z, memory)) @ memory
  update    : per-slot masked softmax over argmax rows -> scatter into memory,
              renormalize touched slots.

Distribution: data-parallel over B across 8 NeuronCores; per-slot partial
update/denominator ReduceScattered over cores; each core renormalizes its
N-shard. Math notes:
  * softmax shift per update column is mathematically free; we use shift 0
    (values exp(max_sim) are in [1/e, e], no overflow), so no cross-core
    column-max pass is needed.
  * l2norm(mem + upd/denom) == l2norm(denom*mem + upd) for denom > 0, which
    lets the denominator division fold into the final renormalize.
"""
import sys

sys.path.insert(0, "/opt/trn_rl_repo")

import os

import numpy as np

import concourse.bass as bass
import concourse.mybir as mybir
import concourse.tile as tile
from concourse.bass_utils import run_bass_kernel_spmd
from concourse.masks import make_identity

B, N, H = 32768, 2048, 512
NCORES = 8
P = 128
BSH = B // NCORES            # 4096 rows per core
CH = BSH // P                # 32 chunks per core
NT = N // P                  # 16 N tiles
KH = H // P                  # 4 H (contraction) chunks
NSH = N // NCORES            # 256 slots per core
NTS = NSH // P               # 2 N-shard tiles

f32 = mybir.dt.float32
f32r = mybir.dt.float32r
bf16 = mybir.dt.bfloat16
AF = mybir.ActivationFunctionType
OP = mybir.AluOpType

# Similarity matmul dtype: f32r (TF32-like, 1 cyc/row) vs f32 (exact, 4 cyc/row).
PRECISE_SIM = False


def _split_multi_waits(nc):
    """This walrus build accepts one sync-wait slot per instruction; hoist
    extra waits emitted by Tile onto same-engine NOPs placed just before."""
    for bb in nc.main_func.blocks:
        new = []
        dirty = False
        for ins in bb.instructions:
            si = ins.sync_info
            if si is not None and len(si.on_wait) > 1:
                waits = list(si.on_wait)
                for k, w in enumerate(waits[:-1]):
                    nop = mybir.InstNoOp(name=f"{ins.name}.w{k}", ins=[], outs=[])
                    nop.engine = ins.engine
                    nop.sync_info = mybir.SyncInfo(on_wait=[w], on_update=[])
                    new.append(nop)
                ins.sync_info = mybir.SyncInfo(
                    on_wait=[waits[-1]], on_update=list(si.on_update)
                )
                new.append(ins)
                dirty = True
            else:
                new.append(ins)
        if dirty:
            bb.instructions = new


def build():
    nc = bass.Bass(target_bir_lowering=False)

    z_sh = nc.declare_dram_parameter("z_sh", [BSH, H], f32, isOutput=False)
    mem_in = nc.declare_dram_parameter("mem_in", [N, H], f32, isOutput=False)
    mem_shard = nc.declare_dram_parameter("mem_shard", [NSH, H], f32, isOutput=False)
    zhat_sh = nc.declare_dram_parameter("zhat_sh", [BSH, H], f32, isOutput=True)
    newmem_sh = nc.declare_dram_parameter("newmem_sh", [NSH, H], f32, isOutput=True)
    dbg_den = nc.declare_dram_parameter("dbg_den", [N], f32, isOutput=True)
    dbg_upd = nc.declare_dram_parameter("dbg_upd", [N, H], f32, isOutput=True)
    dbg_denrs = nc.declare_dram_parameter("dbg_denrs", [NSH], f32, isOutput=True)

    # internal DRAM
    stash = nc.dram_tensor("stash", [NT, CH, P, P], bf16)      # u blocks
    upd_loc = nc.dram_tensor("upd_loc", [N, H], f32)
    upd_rs = nc.dram_tensor("upd_rs", [NSH, H], f32)
    den_loc = nc.dram_tensor("den_loc", [N], f32)
    den_rs = nc.dram_tensor("den_rs", [NSH], f32)

    rg = [list(range(NCORES))]

    with tile.TileContext(nc) as tc:
        with (
            tc.tile_pool(name="const", bufs=1) as const,
            tc.tile_pool(name="resident", bufs=1) as res,
            tc.tile_pool(name="work", bufs=2) as work,
            tc.tile_pool(name="small", bufs=3) as small,
        ):
            from contextlib import ExitStack
            p1ctx = ExitStack()
            ps_sim = p1ctx.enter_context(tc.tile_pool(name="ps_sim", bufs=1, space="PSUM"))
            ps_tr = p1ctx.enter_context(tc.tile_pool(name="ps_tr", bufs=1, space="PSUM"))
            ps_zh = p1ctx.enter_context(tc.tile_pool(name="ps_zh", bufs=1, space="PSUM"))
            # ---------------- setup ----------------
            ident = const.tile([P, P], f32)
            make_identity(nc, ident[:])
            identb = const.tile([P, P], bf16)
            make_identity(nc, identb[:])
            ones_b = const.tile([P, 1], bf16)
            nc.vector.memset(ones_b[:], 1.0)

            # absorb gpsimd sem on PE so later transposes carry 1 wait
            dps = ps_tr.tile([P, P], f32, tag="tr")
            nc.tensor.transpose(dps[:], ident[:], ident[:])

            mem = res.tile([P, NT, H], f32)       # raw memory, tile nt = rows
            mem_b = res.tile([P, NT, H], bf16)    # bf16 copy for z_hat matmul
            nc.sync.dma_start(mem[:], mem_in.rearrange("(t p) h -> p t h", p=P))
            nc.vector.tensor_copy(mem_b[:], mem[:])

            # memory row norms -> normalized copy (f32r) -> transpose to mT
            msq = small.tile([P, NT], f32, tag="msq")
            for nt in range(NT):
                junk = work.tile([P, H], f32, tag="sqj")
                nc.scalar.activation(
                    junk[:], mem[:, nt], AF.Square, accum_out=msq[:, nt : nt + 1]
                )
            mnorm = small.tile([P, NT], f32, tag="mnorm")
            nc.scalar.activation(mnorm[:], msq[:], AF.Sqrt)
            mnc = small.tile([P, NT], f32, tag="mnc")
            nc.vector.tensor_scalar_max(mnc[:], mnorm[:], 1e-12)
            mrn = small.tile([P, NT], f32, tag="mrn")
            nc.vector.reciprocal(mrn[:], mnc[:])

            mT = res.tile([P, KH, N], f32r)       # m_n.T : [H-part, k, N]
            # raw mem f32 is only needed for mem_b/msq; normalize in place
            for nt in range(NT):
                nc.vector.tensor_scalar_mul(
                    mem[:, nt], mem[:, nt], mrn[:, nt : nt + 1]
                )
            for nt in range(NT):
                for k in range(KH):
                    tp = ps_tr.tile([P, P], f32, tag="tr")
                    nc.tensor.transpose(
                        tp[:], mem[:, nt, k * P : (k + 1) * P], ident[:]
                    )
                    if (nt + k) % 2 == 0:
                        nc.vector.tensor_copy(
                            mT[:, k, nt * P : (nt + 1) * P], tp[:]
                        )
                    else:
                        nc.scalar.activation(
                            mT[:, k, nt * P : (nt + 1) * P], tp[:], AF.Copy
                        )

            z_bf = res.tile([P, CH, H], bf16)     # bf16 z kept for pass 2

            # ---------------- pass 1 ----------------
            for c in range(CH):
                z_c = work.tile([P, H], f32, tag="z_c")
                nc.sync.dma_start(z_c[:], z_sh[c * P : (c + 1) * P, :])

                ssq = small.tile([P, 1], f32, tag="ssq")
                junk = work.tile([P, H], f32, tag="sqj")
                nc.scalar.activation(junk[:], z_c[:], AF.Square, accum_out=ssq[:])
                znorm = small.tile([P, 1], f32, tag="znorm")
                nc.scalar.activation(znorm[:], ssq[:], AF.Sqrt)
                znc = small.tile([P, 1], f32, tag="znc")
                nc.vector.tensor_scalar_max(znc[:], znorm[:], 1e-12)
                rnorm = small.tile([P, 1], f32, tag="rnorm")
                nc.vector.reciprocal(rnorm[:], znc[:])

                # z chunk transposed -> lhsT for similarity
                zT = work.tile([P, KH, P], f32r if not PRECISE_SIM else f32,
                               tag="zT")
                for k in range(KH):
                    tp = ps_tr.tile([P, P], f32, tag="tr")
                    nc.tensor.transpose(tp[:], z_c[:, k * P : (k + 1) * P], ident[:])
                    if k % 2 == 0:
                        nc.vector.tensor_copy(zT[:, k], tp[:])
                    else:
                        nc.scalar.activation(zT[:, k], tp[:], AF.Copy)

                # similarity: sim_raw = z . m_n  (rows of z unnormalized)
                simp = ps_sim.tile([P, N], f32, tag="simp")
                for k in range(KH):
                    for n4 in range(4):
                        nc.tensor.matmul(
                            simp[:, n4 * 512 : (n4 + 1) * 512],
                            zT[:, k],
                            mT[:, k, n4 * 512 : (n4 + 1) * 512],
                            start=(k == 0),
                            stop=(k == KH - 1),
                        )

                sim = work.tile([P, N], f32, tag="sim")
                for n4 in range(4):
                    sl = slice(n4 * 512, (n4 + 1) * 512)
                    if n4 % 2 == 0:
                        nc.vector.tensor_copy(sim[:, sl], simp[:, sl])
                    else:
                        nc.scalar.activation(sim[:, sl], simp[:, sl], AF.Copy)

                M = small.tile([P, 1], f32, tag="M")
                nc.vector.reduce_max(M[:], sim[:], axis=mybir.AxisListType.X)
                negMr = small.tile([P, 1], f32, tag="negMr")
                nc.vector.tensor_scalar(
                    negMr[:], M[:], rnorm[:], -1.0, OP.mult, OP.mult
                )
                expM = small.tile([P, 1], f32, tag="expM")
                nc.scalar.activation(expM[:], M[:], AF.Exp, scale=rnorm[:])

                expb = work.tile([P, N], bf16, tag="expb")
                sumexp = small.tile([P, 1], f32, tag="sumexp")
                nc.scalar.activation(
                    expb[:], sim[:], AF.Exp, bias=negMr[:], scale=rnorm[:],
                    accum_out=sumexp[:],
                )

                u_bf = work.tile([P, N], bf16, tag="u_bf")
                nc.vector.tensor_scalar(
                    u_bf[:], sim[:], M[:], expM[:], OP.is_ge, OP.mult
                )
                nc.sync.dma_start(
                    stash[:, c].rearrange("t p f -> p t f"),
                    u_bf[:].rearrange("p (t f) -> p t f", f=P),
                )

                # z_hat = (expb @ mem) * (1/sumexp)
                zhp = ps_zh.tile([P, H], f32, tag="zhp")
                for nt in range(NT):
                    tp = ps_tr.tile([P, P], bf16, tag="tr_b")
                    nc.tensor.transpose(
                        tp[:], expb[:, nt * P : (nt + 1) * P], identb[:]
                    )
                    eT = work.tile([P, P], bf16, tag="eT")
                    if nt % 2 == 0:
                        nc.vector.tensor_copy(eT[:], tp[:])
                    else:
                        nc.scalar.activation(eT[:], tp[:], AF.Copy)
                    nc.tensor.matmul(
                        zhp[:], eT[:], mem_b[:, nt], start=(nt == 0),
                        stop=(nt == NT - 1),
                    )
                rsum = small.tile([P, 1], f32, tag="rsum")
                nc.vector.reciprocal(rsum[:], sumexp[:])
                zh = work.tile([P, H], f32, tag="zh")
                nc.scalar.activation(zh[:], zhp[:], AF.Copy, scale=rsum[:])
                nc.sync.dma_start(zhat_sh[c * P : (c + 1) * P, :], zh[:])

                nc.vector.tensor_copy(z_bf[:, c], z_c[:])

            p1ctx.close()
            # ---------------- pass 2: update + denom ----------------
            with (
                tc.tile_pool(name="ps_upd", bufs=2, space="PSUM") as ps_upd,
                tc.tile_pool(name="ps_den", bufs=1, space="PSUM") as ps_den,
            ):
                denp = ps_den.tile([P, NT], f32)
                for nt in range(NT):
                    updp = ps_upd.tile([P, H], f32, tag="updp")
                    for c in range(CH):
                        u_t = work.tile([P, P], bf16, tag="u_t")
                        nc.sync.dma_start(u_t[:], stash[nt, c])
                        nc.tensor.matmul(
                            updp[:], u_t[:], z_bf[:, c], start=(c == 0),
                            stop=(c == CH - 1),
                        )
                        nc.tensor.matmul(
                            denp[:, nt : nt + 1], u_t[:], ones_b[:],
                            start=(c == 0), stop=(c == CH - 1),
                        )
                    us = work.tile([P, H], f32, tag="us")
                    nc.scalar.activation(us[:], updp[:], AF.Copy)
                    nc.sync.dma_start(upd_loc[nt * P : (nt + 1) * P, :], us[:])
                dsb = small.tile([P, NT], f32, tag="dsb")
                nc.vector.tensor_copy(dsb[:], denp[:])
                nc.sync.dma_start(
                    den_loc.rearrange("(t p) -> p t", p=P), dsb[:]
                )

            # ---------------- reduce-scatter + renormalize shard ----------
            nc.gpsimd.collective_compute(
                "ReduceScatter", OP.add, replica_groups=rg,
                ins=[upd_loc[:]], outs=[upd_rs[:]],
            )
            nc.gpsimd.collective_compute(
                "ReduceScatter", OP.add, replica_groups=rg,
                ins=[den_loc[:]], outs=[den_rs[:]],
            )

            # debug taps
            dcp = work.tile([P, NT], f32, tag="dcp")
            nc.sync.dma_start(dcp[:], den_loc.rearrange("(t p) -> p t", p=P))
            nc.sync.dma_start(dbg_den.rearrange("(t p) -> p t", p=P), dcp[:])
            for nt in range(NT):
                ucp = work.tile([P, H], f32, tag="ucp")
                nc.sync.dma_start(ucp[:], upd_loc[nt * P : (nt + 1) * P, :])
                nc.sync.dma_start(dbg_upd[nt * P : (nt + 1) * P, :], ucp[:])
            dcp2 = work.tile([P, NTS], f32, tag="dcp2")
            nc.sync.dma_start(dcp2[:], den_rs.rearrange("(t p) -> p t", p=P))
            nc.sync.dma_start(dbg_denrs.rearrange("(t p) -> p t", p=P), dcp2[:])

            dsh = small.tile([P, NTS], f32, tag="dsh")
            nc.sync.dma_start(dsh[:], den_rs.rearrange("(t p) -> p t", p=P))
            for t in range(NTS):
                mt = work.tile([P, H], f32, tag="mt")
                nc.sync.dma_start(mt[:], mem_shard[t * P : (t + 1) * P, :])
                ut = work.tile([P, H], f32, tag="ut")
                nc.sync.dma_start(ut[:], upd_rs[t * P : (t + 1) * P, :])

                dcol = dsh[:, t : t + 1]
                sel = small.tile([P, 1], f32, tag="sel")
                nc.vector.tensor_scalar(sel[:], dcol, 0.5, None, OP.is_ge)
                inv = small.tile([P, 1], f32, tag="inv")  # 1 - sel
                nc.vector.tensor_scalar(inv[:], sel[:], -1.0, 1.0, OP.mult, OP.add)

                nm = work.tile([P, H], f32, tag="nm")
                nc.vector.tensor_scalar_mul(nm[:], mt[:], dcol)     # denom*mem
                nc.vector.tensor_add(nm[:], nm[:], ut[:])           # + upd
                keep = work.tile([P, H], f32, tag="keep")
                nc.vector.tensor_scalar_mul(keep[:], mt[:], inv[:])
                nc.vector.tensor_add(nm[:], nm[:], keep[:])

                sq = small.tile([P, 1], f32, tag="sq2")
                junk2 = work.tile([P, H], f32, tag="sqj")
                nc.scalar.activation(junk2[:], nm[:], AF.Square, accum_out=sq[:])
                nrm = small.tile([P, 1], f32, tag="nrm")
                nc.scalar.activation(nrm[:], sq[:], AF.Sqrt)
                nrmc = small.tile([P, 1], f32, tag="nrmc")
                nc.vector.tensor_scalar_max(nrmc[:], nrm[:], 1e-12)
                # divisor = sel*norm + (1-sel)
                dv = small.tile([P, 1], f32, tag="dv")
                nc.vector.tensor_scalar_mul(dv[:], nrmc[:], sel[:])
                nc.vector.tensor_add(dv[:], dv[:], inv[:])
                rdv = small.tile([P, 1], f32, tag="rdv")
                nc.vector.reciprocal(rdv[:], dv[:])
                out_t = work.tile([P, H], f32, tag="out_t")
                nc.vector.tensor_scalar_mul(out_t[:], nm[:], rdv[:])
                nc.sync.dma_start(newmem_sh[t * P : (t + 1) * P, :], out_t[:])

    _split_multi_waits(nc)
    return nc


_NC_CACHE = None
LAST_RESULT = None


def kernel(z: np.ndarray, memory_items: np.ndarray):
    global _NC_CACHE
    if _NC_CACHE is None:
        _NC_CACHE = build()
    nc = _NC_CACHE

    z = np.ascontiguousarray(z, dtype=np.float32)
    mem = np.ascontiguousarray(memory_items, dtype=np.float32)
    in_maps = []
    for c in range(NCORES):
        in_maps.append(
            {
                "z_sh": z[c * BSH : (c + 1) * BSH],
                "mem_in": mem,
                "mem_shard": mem[c * NSH : (c + 1) * NSH],
            }
        )
    trace = bool(os.environ.get("KER_TRACE"))
    res = run_bass_kernel_spmd(
        nc, in_maps, list(range(NCORES)), trace=trace,
        trace_cores=[0] if trace else None,
    )
    global LAST_RESULT
    LAST_RESULT = res
    z_hat = np.concatenate([res.results[c]["zhat_sh"] for c in range(NCORES)], axis=0)
    new_mem = np.concatenate(
        [res.results[c]["newmem_sh"] for c in range(NCORES)], axis=0
    )
    return z_hat, new_mem


if __name__ == "__main__":
    rng = np.random.default_rng(0)
    z = rng.standard_normal((B, H)).astype(np.float32)
    m = rng.standard_normal((N, H)).astype(np.float32)
    zh, nm = kernel(z=z, memory_items=m)
    print(zh.shape, nm.shape, np.abs(zh).max(), np.abs(nm).max())


# revision 14
# speedup vs baseline: 1.1177x; 1.1177x over previous
"""Trainium2 Bass kernel for nn_MemoryModule (scatter_memory).

Computes, for z (B,H) and memory_items (N,H):
  read path : z_hat = softmax(cos_sim(z, memory)) @ memory
  update    : per-slot masked softmax over argmax rows -> scatter into memory,
              renormalize touched slots.

Distribution: data-parallel over B across 8 NeuronCores; per-slot partial
update/denominator ReduceScattered over cores; each core renormalizes its
N-shard. Math notes:
  * softmax shift per update column is mathematically free; we use shift 0
    (values exp(max_sim) are in [1/e, e], no overflow), so no cross-core
    column-max pass is needed.
  * l2norm(mem + upd/denom) == l2norm(denom*mem + upd) for denom > 0, which
    lets the denominator division fold into the final renormalize.
"""
import sys

sys.path.insert(0, "/opt/trn_rl_repo")

import os

import numpy as np

import concourse.bass as bass
import concourse.mybir as mybir
import concourse.tile as tile
from concourse.bass_utils import run_bass_kernel_spmd
from concourse.masks import make_identity

B, N, H = 32768, 2048, 512
NCORES = 8
P = 128
BSH = B // NCORES            # 4096 rows per core
CH = BSH // P                # 32 chunks per core
NT = N // P                  # 16 N tiles
KH = H // P                  # 4 H (contraction) chunks
NSH = N // NCORES            # 256 slots per core
NTS = NSH // P               # 2 N-shard tiles

f32 = mybir.dt.float32
f32r = mybir.dt.float32r
bf16 = mybir.dt.bfloat16
AF = mybir.ActivationFunctionType
OP = mybir.AluOpType

# Similarity matmul dtype: f32r (TF32-like, 1 cyc/row) vs f32 (exact, 4 cyc/row).
PRECISE_SIM = False


def _split_multi_waits(nc):
    """This walrus build accepts one sync-wait slot per instruction; hoist
    extra waits emitted by Tile onto same-engine NOPs placed just before."""
    for bb in nc.main_func.blocks:
        new = []
        dirty = False
        for ins in bb.instructions:
            si = ins.sync_info
            if si is not None and len(si.on_wait) > 1:
                waits = list(si.on_wait)
                for k, w in enumerate(waits[:-1]):
                    nop = mybir.InstNoOp(name=f"{ins.name}.w{k}", ins=[], outs=[])
                    nop.engine = ins.engine
                    nop.sync_info = mybir.SyncInfo(on_wait=[w], on_update=[])
                    new.append(nop)
                ins.sync_info = mybir.SyncInfo(
                    on_wait=[waits[-1]], on_update=list(si.on_update)
                )
                new.append(ins)
                dirty = True
            else:
                new.append(ins)
        if dirty:
            bb.instructions = new


def build():
    nc = bass.Bass(target_bir_lowering=False)

    z_sh = nc.declare_dram_parameter("z_sh", [BSH, H], f32, isOutput=False)
    mem_in = nc.declare_dram_parameter("mem_in", [N, H], f32, isOutput=False)
    mem_shard = nc.declare_dram_parameter("mem_shard", [NSH, H], f32, isOutput=False)
    zhat_sh = nc.declare_dram_parameter("zhat_sh", [BSH, H], f32, isOutput=True)
    newmem_sh = nc.declare_dram_parameter("newmem_sh", [NSH, H], f32, isOutput=True)
    dbg_den = nc.declare_dram_parameter("dbg_den", [N], f32, isOutput=True)
    dbg_upd = nc.declare_dram_parameter("dbg_upd", [N, H], f32, isOutput=True)
    dbg_denrs = nc.declare_dram_parameter("dbg_denrs", [NSH], f32, isOutput=True)

    # internal DRAM
    stash = nc.dram_tensor("stash", [NT, CH, P, P], bf16)      # u blocks
    upd_loc = nc.dram_tensor("upd_loc", [N, H], f32)
    upd_rs = nc.dram_tensor("upd_rs", [NSH, H], f32)
    den_loc = nc.dram_tensor("den_loc", [N], f32)
    den_rs = nc.dram_tensor("den_rs", [NSH], f32)

    rg = [list(range(NCORES))]

    with tile.TileContext(nc) as tc:
        with (
            tc.tile_pool(name="const", bufs=1) as const,
            tc.tile_pool(name="resident", bufs=1) as res,
            tc.tile_pool(name="work", bufs=2) as work,
            tc.tile_pool(name="small", bufs=3) as small,
        ):
            from contextlib import ExitStack
            p1ctx = ExitStack()
            ps_sim = p1ctx.enter_context(tc.tile_pool(name="ps_sim", bufs=2, space="PSUM"))
            ps_tr = p1ctx.enter_context(tc.tile_pool(name="ps_tr", bufs=2, space="PSUM"))
            ps_etr = p1ctx.enter_context(tc.tile_pool(name="ps_etr", bufs=1, space="PSUM"))
            ps_zh = p1ctx.enter_context(tc.tile_pool(name="ps_zh", bufs=1, space="PSUM"))
            # ---------------- setup ----------------
            ident = const.tile([P, P], f32)
            make_identity(nc, ident[:])
            identb = const.tile([P, P], bf16)
            make_identity(nc, identb[:])
            ones_b = const.tile([P, 1], bf16)
            nc.vector.memset(ones_b[:], 1.0)

            # absorb gpsimd sem on PE so later transposes carry 1 wait
            dps = ps_tr.tile([P, P], f32, tag="tr")
            nc.tensor.transpose(dps[:], ident[:], ident[:])

            mem = res.tile([P, NT, H], f32)       # raw memory, tile nt = rows
            mem_b = res.tile([P, NT, H], bf16)    # bf16 copy for z_hat matmul
            nc.sync.dma_start(mem[:], mem_in.rearrange("(t p) h -> p t h", p=P))
            nc.vector.tensor_copy(mem_b[:], mem[:])

            # memory row norms -> normalized copy (f32r) -> transpose to mT
            msq = small.tile([P, NT], f32, tag="msq")
            for nt in range(NT):
                junk = work.tile([P, H], f32, tag="sqj")
                nc.scalar.activation(
                    junk[:], mem[:, nt], AF.Square, accum_out=msq[:, nt : nt + 1]
                )
            mnorm = small.tile([P, NT], f32, tag="mnorm")
            nc.scalar.activation(mnorm[:], msq[:], AF.Sqrt)
            mnc = small.tile([P, NT], f32, tag="mnc")
            nc.vector.tensor_scalar_max(mnc[:], mnorm[:], 1e-12)
            mrn = small.tile([P, NT], f32, tag="mrn")
            nc.vector.reciprocal(mrn[:], mnc[:])

            mT = res.tile([P, KH, N], f32r)       # m_n.T : [H-part, k, N]
            # raw mem f32 is only needed for mem_b/msq; normalize in place
            for nt in range(NT):
                nc.vector.tensor_scalar_mul(
                    mem[:, nt], mem[:, nt], mrn[:, nt : nt + 1]
                )
            for nt in range(NT):
                for k in range(KH):
                    tp = ps_tr.tile([P, P], f32, tag="tr")
                    nc.tensor.transpose(
                        tp[:], mem[:, nt, k * P : (k + 1) * P], ident[:]
                    )
                    if (nt + k) % 2 == 0:
                        nc.vector.tensor_copy(
                            mT[:, k, nt * P : (nt + 1) * P], tp[:]
                        )
                    else:
                        nc.scalar.activation(
                            mT[:, k, nt * P : (nt + 1) * P], tp[:], AF.Copy
                        )

            z_bf = res.tile([P, CH, H], bf16)     # bf16 z kept for pass 2

            # ---------------- prologue: all z row norms ----------------
            # (batched so the steady-state ACT runs Exp only, no table swaps)
            ssq_all = small.tile([P, CH], f32, tag="ssq_all")
            for c in range(CH):
                zp = work.tile([P, H], f32, tag="z_c")
                nc.sync.dma_start(zp[:], z_sh[c * P : (c + 1) * P, :])
                junk = work.tile([P, H], f32, tag="sqj")
                nc.scalar.activation(
                    junk[:], zp[:], AF.Square, accum_out=ssq_all[:, c : c + 1]
                )
            znorm_all = small.tile([P, CH], f32, tag="znorm_all")
            nc.scalar.activation(znorm_all[:], ssq_all[:], AF.Sqrt)
            znc_all = small.tile([P, CH], f32, tag="znc_all")
            nc.vector.tensor_scalar_max(znc_all[:], znorm_all[:], 1e-12)
            rnorm_all = res.tile([P, CH], f32)
            nc.vector.reciprocal(rnorm_all[:], znc_all[:])

            # ---------------- pass 1 ----------------
            for c in range(CH):
                rnorm = rnorm_all[:, c : c + 1]
                z_c = work.tile([P, H], f32, tag="z_c")
                nc.sync.dma_start(z_c[:], z_sh[c * P : (c + 1) * P, :])

                # z chunk transposed -> lhsT for similarity
                zT = work.tile([P, KH, P], f32r if not PRECISE_SIM else f32,
                               tag="zT")
                for k in range(KH):
                    tp = ps_tr.tile([P, P], f32, tag="tr")
                    nc.tensor.transpose(tp[:], z_c[:, k * P : (k + 1) * P], ident[:])
                    nc.vector.tensor_copy(zT[:, k], tp[:])

                # similarity: sim_raw = z . m_n  (rows of z unnormalized)
                sim = work.tile([P, N], f32, tag="sim")
                for n4 in range(4):
                    simp = ps_sim.tile([P, 512], f32, tag="simp")
                    for k in range(KH):
                        nc.tensor.matmul(
                            simp[:],
                            zT[:, k],
                            mT[:, k, n4 * 512 : (n4 + 1) * 512],
                            start=(k == 0),
                            stop=(k == KH - 1),
                        )
                    nc.vector.tensor_copy(sim[:, n4 * 512 : (n4 + 1) * 512], simp[:])

                M = small.tile([P, 1], f32, tag="M")
                nc.vector.reduce_max(M[:], sim[:], axis=mybir.AxisListType.X)
                negMr = small.tile([P, 1], f32, tag="negMr")
                nc.vector.tensor_scalar(
                    negMr[:], M[:], rnorm, -1.0, OP.mult, OP.mult
                )
                expM = small.tile([P, 1], f32, tag="expM")
                nc.scalar.activation(expM[:], M[:], AF.Exp, scale=rnorm)

                expb = work.tile([P, N], bf16, tag="expb")
                sumexp = small.tile([P, 1], f32, tag="sumexp")
                nc.scalar.activation(
                    expb[:], sim[:], AF.Exp, bias=negMr[:], scale=rnorm,
                    accum_out=sumexp[:],
                )

                u_bf = work.tile([P, N], bf16, tag="u_bf")
                nc.vector.tensor_scalar(
                    u_bf[:], sim[:], M[:], expM[:], OP.is_ge, OP.mult
                )
                nc.sync.dma_start(
                    stash[:, c].rearrange("t p f -> p t f"),
                    u_bf[:].rearrange("p (t f) -> p t f", f=P),
                )

                # z_hat = (expb @ mem) * (1/sumexp)
                zhp = ps_zh.tile([P, H], f32, tag="zhp")
                eTp = ps_etr.tile([P, N], bf16, tag="etr")
                eT = work.tile([P, N], bf16, tag="eT")
                for half in range(2):
                    for j in range(8):
                        nt = half * 8 + j
                        nc.tensor.transpose(
                            eTp[:, nt * P : (nt + 1) * P],
                            expb[:, nt * P : (nt + 1) * P],
                            identb[:],
                        )
                    hs = slice(half * 1024, (half + 1) * 1024)
                    nc.scalar.activation(eT[:, hs], eTp[:, hs], AF.Copy)
                    for j in range(8):
                        nt = half * 8 + j
                        nc.tensor.matmul(
                            zhp[:], eT[:, nt * P : (nt + 1) * P], mem_b[:, nt],
                            start=(nt == 0), stop=(nt == NT - 1),
                        )
                rsum = small.tile([P, 1], f32, tag="rsum")
                nc.vector.reciprocal(rsum[:], sumexp[:])
                zh = work.tile([P, H], f32, tag="zh")
                nc.vector.tensor_scalar_mul(zh[:], zhp[:], rsum[:])
                nc.sync.dma_start(zhat_sh[c * P : (c + 1) * P, :], zh[:])

                nc.vector.tensor_copy(z_bf[:, c], z_c[:])

            p1ctx.close()
            # ---------------- pass 2: update + denom ----------------
            with (
                tc.tile_pool(name="ps_upd", bufs=2, space="PSUM") as ps_upd,
                tc.tile_pool(name="ps_den", bufs=1, space="PSUM") as ps_den,
            ):
                denp = ps_den.tile([P, NT], f32)
                for nt in range(NT):
                    updp = ps_upd.tile([P, H], f32, tag="updp")
                    for c in range(CH):
                        u_t = work.tile([P, P], bf16, tag="u_t")
                        nc.sync.dma_start(u_t[:], stash[nt, c])
                        nc.tensor.matmul(
                            updp[:], u_t[:], z_bf[:, c], start=(c == 0),
                            stop=(c == CH - 1),
                        )
                        nc.tensor.matmul(
                            denp[:, nt : nt + 1], u_t[:], ones_b[:],
                            start=(c == 0), stop=(c == CH - 1),
                        )
                    us = work.tile([P, H], f32, tag="us")
                    nc.scalar.activation(us[:], updp[:], AF.Copy)
                    nc.sync.dma_start(upd_loc[nt * P : (nt + 1) * P, :], us[:])
                dsb = small.tile([P, NT], f32, tag="dsb")
                nc.vector.tensor_copy(dsb[:], denp[:])
                nc.sync.dma_start(
                    den_loc.rearrange("(t p) -> p t", p=P), dsb[:]
                )

            # ---------------- reduce-scatter + renormalize shard ----------
            nc.gpsimd.collective_compute(
                "ReduceScatter", OP.add, replica_groups=rg,
                ins=[upd_loc[:]], outs=[upd_rs[:]],
            )
            nc.gpsimd.collective_compute(
                "ReduceScatter", OP.add, replica_groups=rg,
                ins=[den_loc[:]], outs=[den_rs[:]],
            )

            # debug taps
            dcp = work.tile([P, NT], f32, tag="dcp")
            nc.sync.dma_start(dcp[:], den_loc.rearrange("(t p) -> p t", p=P))
            nc.sync.dma_start(dbg_den.rearrange("(t p) -> p t", p=P), dcp[:])
            for nt in range(NT):
                ucp = work.tile([P, H], f32, tag="ucp")
                nc.sync.dma_start(ucp[:], upd_loc[nt * P : (nt + 1) * P, :])
                nc.sync.dma_start(dbg_upd[nt * P : (nt + 1) * P, :], ucp[:])
            dcp2 = work.tile([P, NTS], f32, tag="dcp2")
            nc.sync.dma_start(dcp2[:], den_rs.rearrange("(t p) -> p t", p=P))
            nc.sync.dma_start(dbg_denrs.rearrange("(t p) -> p t", p=P), dcp2[:])

            dsh = small.tile([P, NTS], f32, tag="dsh")
            nc.sync.dma_start(dsh[:], den_rs.rearrange("(t p) -> p t", p=P))
            for t in range(NTS):
                mt = work.tile([P, H], f32, tag="mt")
                nc.sync.dma_start(mt[:], mem_shard[t * P : (t + 1) * P, :])
                ut = work.tile([P, H], f32, tag="ut")
                nc.sync.dma_start(ut[:], upd_rs[t * P : (t + 1) * P, :])

                dcol = dsh[:, t : t + 1]
                sel = small.tile([P, 1], f32, tag="sel")
                nc.vector.tensor_scalar(sel[:], dcol, 0.5, None, OP.is_ge)
                inv = small.tile([P, 1], f32, tag="inv")  # 1 - sel
                nc.vector.tensor_scalar(inv[:], sel[:], -1.0, 1.0, OP.mult, OP.add)

                nm = work.tile([P, H], f32, tag="nm")
                nc.vector.tensor_scalar_mul(nm[:], mt[:], dcol)     # denom*mem
                nc.vector.tensor_add(nm[:], nm[:], ut[:])           # + upd
                keep = work.tile([P, H], f32, tag="keep")
                nc.vector.tensor_scalar_mul(keep[:], mt[:], inv[:])
                nc.vector.tensor_add(nm[:], nm[:], keep[:])

                sq = small.tile([P, 1], f32, tag="sq2")
                junk2 = work.tile([P, H], f32, tag="sqj")
                nc.scalar.activation(junk2[:], nm[:], AF.Square, accum_out=sq[:])
                nrm = small.tile([P, 1], f32, tag="nrm")
                nc.scalar.activation(nrm[:], sq[:], AF.Sqrt)
                nrmc = small.tile([P, 1], f32, tag="nrmc")
                nc.vector.tensor_scalar_max(nrmc[:], nrm[:], 1e-12)
                # divisor = sel*norm + (1-sel)
                dv = small.tile([P, 1], f32, tag="dv")
                nc.vector.tensor_scalar_mul(dv[:], nrmc[:], sel[:])
                nc.vector.tensor_add(dv[:], dv[:], inv[:])
                rdv = small.tile([P, 1], f32, tag="rdv")
                nc.vector.reciprocal(rdv[:], dv[:])
                out_t = work.tile([P, H], f32, tag="out_t")
                nc.vector.tensor_scalar_mul(out_t[:], nm[:], rdv[:])
                nc.sync.dma_start(newmem_sh[t * P : (t + 1) * P, :], out_t[:])

    _split_multi_waits(nc)
    return nc


_NC_CACHE = None
LAST_RESULT = None


def kernel(z: np.ndarray, memory_items: np.ndarray):
    global _NC_CACHE
    if _NC_CACHE is None:
        _NC_CACHE = build()
    nc = _NC_CACHE

    z = np.ascontiguousarray(z, dtype=np.float32)
    mem = np.ascontiguousarray(memory_items, dtype=np.float32)
    in_maps = []
    for c in range(NCORES):
        in_maps.append(
            {
                "z_sh": z[c * BSH : (c + 1) * BSH],
                "mem_in": mem,
                "mem_shard": mem[c * NSH : (c + 1) * NSH],
            }
        )
    trace = bool(os.environ.get("KER_TRACE"))
    res = run_bass_kernel_spmd(
        nc, in_maps, list(range(NCORES)), trace=trace,
        trace_cores=[0] if trace else None,
    )
    global LAST_RESULT
    LAST_RESULT = res
    z_hat = np.concatenate([res.results[c]["zhat_sh"] for c in range(NCORES)], axis=0)
    new_mem = np.concatenate(
        [res.results[c]["newmem_sh"] for c in range(NCORES)], axis=0
    )
    return z_hat, new_mem


if __name__ == "__main__":
    rng = np.random.default_rng(0)
    z = rng.standard_normal((B, H)).astype(np.float32)
    m = rng.standard_normal((N, H)).astype(np.float32)
    zh, nm = kernel(z=z, memory_items=m)
    print(zh.shape, nm.shape, np.abs(zh).max(), np.abs(nm).max())


# revision 17
# speedup vs baseline: 1.7624x; 1.5767x over previous
"""Trainium2 Bass kernel for nn_MemoryModule (scatter_memory).

Computes, for z (B,H) and memory_items (N,H):
  read path : z_hat = softmax(cos_sim(z, memory)) @ memory
  update    : per-slot masked softmax over argmax rows -> scatter into memory,
              renormalize touched slots.

Distribution: data-parallel over B across 8 NeuronCores; per-slot partial
update/denominator ReduceScattered over cores; each core renormalizes its
N-shard. Math notes:
  * softmax shift per update column is mathematically free; we use shift 0
    (values exp(max_sim) are in [1/e, e], no overflow), so no cross-core
    column-max pass is needed.
  * l2norm(mem + upd/denom) == l2norm(denom*mem + upd) for denom > 0, which
    lets the denominator division fold into the final renormalize.
"""
import sys

sys.path.insert(0, "/opt/trn_rl_repo")

import os

import numpy as np

import concourse.bass as bass
import concourse.mybir as mybir
import concourse.tile as tile
from concourse.bass_utils import run_bass_kernel_spmd
from concourse.masks import make_identity

B, N, H = 32768, 2048, 512
NCORES = 8
P = 128
BSH = B // NCORES            # 4096 rows per core
CH = BSH // P                # 32 chunks per core
NT = N // P                  # 16 N tiles
KH = H // P                  # 4 H (contraction) chunks
NSH = N // NCORES            # 256 slots per core
NTS = NSH // P               # 2 N-shard tiles

f32 = mybir.dt.float32
f32r = mybir.dt.float32r
bf16 = mybir.dt.bfloat16
AF = mybir.ActivationFunctionType
OP = mybir.AluOpType

# Similarity matmul dtype: f32r (TF32-like, 1 cyc/row) vs f32 (exact, 4 cyc/row).
PRECISE_SIM = False


def _split_multi_waits(nc):
    """This walrus build accepts one sync-wait slot per instruction; hoist
    extra waits emitted by Tile onto same-engine NOPs placed just before."""
    for bb in nc.main_func.blocks:
        new = []
        dirty = False
        for ins in bb.instructions:
            si = ins.sync_info
            if si is not None and len(si.on_wait) > 1:
                waits = list(si.on_wait)
                for k, w in enumerate(waits[:-1]):
                    nop = mybir.InstNoOp(name=f"{ins.name}.w{k}", ins=[], outs=[])
                    nop.engine = ins.engine
                    nop.sync_info = mybir.SyncInfo(on_wait=[w], on_update=[])
                    new.append(nop)
                ins.sync_info = mybir.SyncInfo(
                    on_wait=[waits[-1]], on_update=list(si.on_update)
                )
                new.append(ins)
                dirty = True
            else:
                new.append(ins)
        if dirty:
            bb.instructions = new


def build():
    nc = bass.Bass(target_bir_lowering=False)

    z_sh = nc.declare_dram_parameter("z_sh", [BSH, H], f32, isOutput=False)
    mem_in = nc.declare_dram_parameter("mem_in", [N, H], f32, isOutput=False)
    mem_shard = nc.declare_dram_parameter("mem_shard", [NSH, H], f32, isOutput=False)
    zhat_sh = nc.declare_dram_parameter("zhat_sh", [BSH, H], f32, isOutput=True)
    newmem_sh = nc.declare_dram_parameter("newmem_sh", [NSH, H], f32, isOutput=True)
    dbg_den = nc.declare_dram_parameter("dbg_den", [N], f32, isOutput=True)
    dbg_upd = nc.declare_dram_parameter("dbg_upd", [N, H], f32, isOutput=True)
    dbg_denrs = nc.declare_dram_parameter("dbg_denrs", [NSH], f32, isOutput=True)

    # internal DRAM
    stash = nc.dram_tensor("stash", [CH, P, N], bf16)         # u rows
    upd_loc = nc.dram_tensor("upd_loc", [N, H], f32)
    upd_rs = nc.dram_tensor("upd_rs", [NSH, H], f32)
    den_loc = nc.dram_tensor("den_loc", [N], f32)
    den_rs = nc.dram_tensor("den_rs", [NSH], f32)

    rg = [list(range(NCORES))]

    with tile.TileContext(nc) as tc:
        with (
            tc.tile_pool(name="const", bufs=1) as const,
            tc.tile_pool(name="resident", bufs=1) as res,
            tc.tile_pool(name="work", bufs=2) as work,
            tc.tile_pool(name="small", bufs=3) as small,
        ):
            from contextlib import ExitStack
            p1ctx = ExitStack()
            ps_sim = p1ctx.enter_context(tc.tile_pool(name="ps_sim", bufs=1, space="PSUM"))
            ps_tr = p1ctx.enter_context(tc.tile_pool(name="ps_tr", bufs=1, space="PSUM"))
            ps_etr = p1ctx.enter_context(tc.tile_pool(name="ps_etr", bufs=1, space="PSUM"))
            ps_zh = p1ctx.enter_context(tc.tile_pool(name="ps_zh", bufs=1, space="PSUM"))
            # ---------------- setup ----------------
            ident = const.tile([P, P], f32)
            make_identity(nc, ident[:])
            identb = const.tile([P, P], bf16)
            make_identity(nc, identb[:])
            ones_b = const.tile([P, 1], bf16)
            nc.vector.memset(ones_b[:], 1.0)

            # absorb gpsimd sem on PE so later transposes carry 1 wait
            dps = ps_tr.tile([P, P], f32, tag="tr")
            nc.tensor.transpose(dps[:], ident[:], ident[:])

            mem = res.tile([P, NT, H], f32)       # raw memory, tile nt = rows
            mem_b = res.tile([P, NT, H], bf16)    # bf16 copy for z_hat matmul
            nc.sync.dma_start(mem[:], mem_in.rearrange("(t p) h -> p t h", p=P))
            nc.vector.tensor_copy(mem_b[:], mem[:])

            # memory row norms -> normalized copy (f32r) -> transpose to mT
            msq = small.tile([P, NT], f32, tag="msq")
            for nt in range(NT):
                junk = work.tile([P, H], f32, tag="sqj")
                nc.scalar.activation(
                    junk[:], mem[:, nt], AF.Square, accum_out=msq[:, nt : nt + 1]
                )
            mnorm = small.tile([P, NT], f32, tag="mnorm")
            nc.scalar.activation(mnorm[:], msq[:], AF.Sqrt)
            mnc = small.tile([P, NT], f32, tag="mnc")
            nc.vector.tensor_scalar_max(mnc[:], mnorm[:], 1e-12)
            mrn = small.tile([P, NT], f32, tag="mrn")
            nc.vector.reciprocal(mrn[:], mnc[:])

            mT = res.tile([P, KH, N], f32r)       # m_n.T : [H-part, k, N]
            # raw mem f32 is only needed for mem_b/msq; normalize in place
            for nt in range(NT):
                nc.vector.tensor_scalar_mul(
                    mem[:, nt], mem[:, nt], mrn[:, nt : nt + 1]
                )
            for nt in range(NT):
                for k in range(KH):
                    tp = ps_tr.tile([P, P], f32, tag="tr")
                    nc.tensor.transpose(
                        tp[:], mem[:, nt, k * P : (k + 1) * P], ident[:]
                    )
                    if (nt + k) % 2 == 0:
                        nc.vector.tensor_copy(
                            mT[:, k, nt * P : (nt + 1) * P], tp[:]
                        )
                    else:
                        nc.scalar.activation(
                            mT[:, k, nt * P : (nt + 1) * P], tp[:], AF.Copy
                        )

            z_bf = res.tile([P, CH, H], bf16)     # bf16 z kept for pass 2

            # ---------------- prologue: all z row norms ----------------
            # (batched so the steady-state ACT runs Exp only, no table swaps)
            ssq_all = small.tile([P, CH], f32, tag="ssq_all")
            for c in range(CH):
                zp = work.tile([P, H], f32, tag="z_c")
                nc.sync.dma_start(zp[:], z_sh[c * P : (c + 1) * P, :])
                junk = work.tile([P, H], f32, tag="sqj")
                nc.scalar.activation(
                    junk[:], zp[:], AF.Square, accum_out=ssq_all[:, c : c + 1]
                )
            znorm_all = small.tile([P, CH], f32, tag="znorm_all")
            nc.scalar.activation(znorm_all[:], ssq_all[:], AF.Sqrt)
            znc_all = small.tile([P, CH], f32, tag="znc_all")
            nc.vector.tensor_scalar_max(znc_all[:], znorm_all[:], 1e-12)
            rnorm_all = res.tile([P, CH], f32)
            nc.vector.reciprocal(rnorm_all[:], znc_all[:])

            # ---------------- pass 1 ----------------
            for c in range(CH):
                rnorm = rnorm_all[:, c : c + 1]
                z_c = work.tile([P, H], f32, tag="z_c")
                nc.sync.dma_start(z_c[:], z_sh[c * P : (c + 1) * P, :])

                # z chunk transposed -> lhsT for similarity
                zT = work.tile([P, KH, P], f32r if not PRECISE_SIM else f32,
                               tag="zT")
                for k in range(KH):
                    tp = ps_tr.tile([P, P], f32, tag="tr")
                    nc.tensor.transpose(tp[:], z_c[:, k * P : (k + 1) * P], ident[:])
                    nc.vector.tensor_copy(zT[:, k], tp[:])

                # similarity: sim_raw = z . m_n  (rows of z unnormalized)
                sim = work.tile([P, N], f32, tag="sim")
                simps = [ps_sim.tile([P, 512], f32, tag=f"simp{n4}", name=f"simp{n4}")
                         for n4 in range(4)]
                for k in range(KH):
                    for n4 in range(4):
                        nc.tensor.matmul(
                            simps[n4][:],
                            zT[:, k],
                            mT[:, k, n4 * 512 : (n4 + 1) * 512],
                            start=(k == 0),
                            stop=(k == KH - 1),
                        )
                for n4 in range(4):
                    nc.vector.tensor_copy(sim[:, n4 * 512 : (n4 + 1) * 512],
                                          simps[n4][:])

                M = small.tile([P, 1], f32, tag="M")
                nc.vector.reduce_max(M[:], sim[:], axis=mybir.AxisListType.X)
                negMr = small.tile([P, 1], f32, tag="negMr")
                nc.vector.tensor_scalar(
                    negMr[:], M[:], rnorm, -1.0, OP.mult, OP.mult
                )
                expM = small.tile([P, 1], f32, tag="expM")
                nc.scalar.activation(expM[:], M[:], AF.Exp, scale=rnorm)

                expb = work.tile([P, N], bf16, tag="expb")
                sumexp = small.tile([P, 1], f32, tag="sumexp")
                nc.scalar.activation(
                    expb[:], sim[:], AF.Exp, bias=negMr[:], scale=rnorm,
                    accum_out=sumexp[:],
                )

                u_bf = work.tile([P, N], bf16, tag="u_bf")
                nc.vector.tensor_scalar(
                    u_bf[:], sim[:], M[:], expM[:], OP.is_ge, OP.mult
                )
                nc.sync.dma_start(stash[c], u_bf[:])

                # z_hat = (expb @ mem) * (1/sumexp)
                zhp = ps_zh.tile([P, H], f32, tag="zhp")
                eTp = ps_etr.tile([P, N], bf16, tag="etr")
                eT = work.tile([P, N], bf16, tag="eT")
                for half in range(2):
                    for j in range(8):
                        nt = half * 8 + j
                        nc.tensor.transpose(
                            eTp[:, nt * P : (nt + 1) * P],
                            expb[:, nt * P : (nt + 1) * P],
                            identb[:],
                        )
                    hs = slice(half * 1024, (half + 1) * 1024)
                    nc.scalar.activation(eT[:, hs], eTp[:, hs], AF.Copy)
                    for j in range(8):
                        nt = half * 8 + j
                        nc.tensor.matmul(
                            zhp[:], eT[:, nt * P : (nt + 1) * P], mem_b[:, nt],
                            start=(nt == 0), stop=(nt == NT - 1),
                        )
                rsum = small.tile([P, 1], f32, tag="rsum")
                nc.vector.reciprocal(rsum[:], sumexp[:])
                zh = work.tile([P, H], f32, tag="zh")
                nc.vector.tensor_scalar_mul(zh[:], zhp[:], rsum[:])
                nc.sync.dma_start(zhat_sh[c * P : (c + 1) * P, :], zh[:])

                nc.vector.tensor_copy(z_bf[:, c], z_c[:])

            p1ctx.close()
            # ---------------- pass 2: update + denom ----------------
            with (
                tc.tile_pool(name="ps_upd", bufs=1, space="PSUM") as ps_upd,
                tc.tile_pool(name="ps_den", bufs=1, space="PSUM") as ps_den,
            ):
                denp = ps_den.tile([P, NT], f32)
                for nt0, nts in ((0, 7), (7, 7), (14, 2)):
                    updps = [ps_upd.tile([P, H], f32, tag=f"updp{j}", name=f"updp{j}")
                             for j in range(nts)]
                    for c in range(CH):
                        band = work.tile([P, nts * P], bf16, tag="band")
                        nc.sync.dma_start(
                            band[:, : nts * P],
                            stash[c][:, nt0 * P : (nt0 + nts) * P],
                        )
                        for j in range(nts):
                            nt = nt0 + j
                            u_t = band[:, j * P : (j + 1) * P]
                            nc.tensor.matmul(
                                updps[j][:], u_t, z_bf[:, c], start=(c == 0),
                                stop=(c == CH - 1),
                            )
                            nc.tensor.matmul(
                                denp[:, nt : nt + 1], u_t, ones_b[:],
                                start=(c == 0), stop=(c == CH - 1),
                            )
                    for j in range(nts):
                        us = work.tile([P, H], f32, tag="us")
                        nc.scalar.activation(us[:], updps[j][:], AF.Copy)
                        nc.sync.dma_start(
                            upd_loc[(nt0 + j) * P : (nt0 + j + 1) * P, :], us[:]
                        )
                dsb = small.tile([P, NT], f32, tag="dsb")
                nc.vector.tensor_copy(dsb[:], denp[:])
                nc.sync.dma_start(
                    den_loc.rearrange("(t p) -> p t", p=P), dsb[:]
                )

            # ---------------- reduce-scatter + renormalize shard ----------
            nc.gpsimd.collective_compute(
                "ReduceScatter", OP.add, replica_groups=rg,
                ins=[upd_loc[:]], outs=[upd_rs[:]],
            )
            nc.gpsimd.collective_compute(
                "ReduceScatter", OP.add, replica_groups=rg,
                ins=[den_loc[:]], outs=[den_rs[:]],
            )

            # debug taps
            dcp = work.tile([P, NT], f32, tag="dcp")
            nc.sync.dma_start(dcp[:], den_loc.rearrange("(t p) -> p t", p=P))
            nc.sync.dma_start(dbg_den.rearrange("(t p) -> p t", p=P), dcp[:])
            for nt in range(NT):
                ucp = work.tile([P, H], f32, tag="ucp")
                nc.sync.dma_start(ucp[:], upd_loc[nt * P : (nt + 1) * P, :])
                nc.sync.dma_start(dbg_upd[nt * P : (nt + 1) * P, :], ucp[:])
            dcp2 = work.tile([P, NTS], f32, tag="dcp2")
            nc.sync.dma_start(dcp2[:], den_rs.rearrange("(t p) -> p t", p=P))
            nc.sync.dma_start(dbg_denrs.rearrange("(t p) -> p t", p=P), dcp2[:])

            dsh = small.tile([P, NTS], f32, tag="dsh")
            nc.sync.dma_start(dsh[:], den_rs.rearrange("(t p) -> p t", p=P))
            for t in range(NTS):
                mt = work.tile([P, H], f32, tag="mt")
                nc.sync.dma_start(mt[:], mem_shard[t * P : (t + 1) * P, :])
                ut = work.tile([P, H], f32, tag="ut")
                nc.sync.dma_start(ut[:], upd_rs[t * P : (t + 1) * P, :])

                dcol = dsh[:, t : t + 1]
                sel = small.tile([P, 1], f32, tag="sel")
                nc.vector.tensor_scalar(sel[:], dcol, 0.5, None, OP.is_ge)
                inv = small.tile([P, 1], f32, tag="inv")  # 1 - sel
                nc.vector.tensor_scalar(inv[:], sel[:], -1.0, 1.0, OP.mult, OP.add)

                nm = work.tile([P, H], f32, tag="nm")
                nc.vector.tensor_scalar_mul(nm[:], mt[:], dcol)     # denom*mem
                nc.vector.tensor_add(nm[:], nm[:], ut[:])           # + upd
                keep = work.tile([P, H], f32, tag="keep")
                nc.vector.tensor_scalar_mul(keep[:], mt[:], inv[:])
                nc.vector.tensor_add(nm[:], nm[:], keep[:])

                sq = small.tile([P, 1], f32, tag="sq2")
                junk2 = work.tile([P, H], f32, tag="sqj")
                nc.scalar.activation(junk2[:], nm[:], AF.Square, accum_out=sq[:])
                nrm = small.tile([P, 1], f32, tag="nrm")
                nc.scalar.activation(nrm[:], sq[:], AF.Sqrt)
                nrmc = small.tile([P, 1], f32, tag="nrmc")
                nc.vector.tensor_scalar_max(nrmc[:], nrm[:], 1e-12)
                # divisor = sel*norm + (1-sel)
                dv = small.tile([P, 1], f32, tag="dv")
                nc.vector.tensor_scalar_mul(dv[:], nrmc[:], sel[:])
                nc.vector.tensor_add(dv[:], dv[:], inv[:])
                rdv = small.tile([P, 1], f32, tag="rdv")
                nc.vector.reciprocal(rdv[:], dv[:])
                out_t = work.tile([P, H], f32, tag="out_t")
                nc.vector.tensor_scalar_mul(out_t[:], nm[:], rdv[:])
                nc.sync.dma_start(newmem_sh[t * P : (t + 1) * P, :], out_t[:])

    _split_multi_waits(nc)
    return nc


_NC_CACHE = None
LAST_RESULT = None


def kernel(z: np.ndarray, memory_items: np.ndarray):
    global _NC_CACHE
    if _NC_CACHE is None:
        _NC_CACHE = build()
    nc = _NC_CACHE

    z = np.ascontiguousarray(z, dtype=np.float32)
    mem = np.ascontiguousarray(memory_items, dtype=np.float32)
    in_maps = []
    for c in range(NCORES):
        in_maps.append(
            {
                "z_sh": z[c * BSH : (c + 1) * BSH],
                "mem_in": mem,
                "mem_shard": mem[c * NSH : (c + 1) * NSH],
            }
        )
    trace = bool(os.environ.get("KER_TRACE"))
    res = run_bass_kernel_spmd(
        nc, in_maps, list(range(NCORES)), trace=trace,
        trace_cores=[0] if trace else None,
    )
    global LAST_RESULT
    LAST_RESULT = res
    z_hat = np.concatenate([res.results[c]["zhat_sh"] for c in range(NCORES)], axis=0)
    new_mem = np.concatenate(
        [res.results[c]["newmem_sh"] for c in range(NCORES)], axis=0
    )
    return z_hat, new_mem


if __name__ == "__main__":
    rng = np.random.default_rng(0)
    z = rng.standard_normal((B, H)).astype(np.float32)
    m = rng.standard_normal((N, H)).astype(np.float32)
    zh, nm = kernel(z=z, memory_items=m)
    print(zh.shape, nm.shape, np.abs(zh).max(), np.abs(nm).max())


# revision 21
# speedup vs baseline: 1.7750x; 1.0071x over previous
"""Trainium2 Bass kernel for nn_MemoryModule (scatter_memory).

Computes, for z (B,H) and memory_items (N,H):
  read path : z_hat = softmax(cos_sim(z, memory)) @ memory
  update    : per-slot masked softmax over argmax rows -> scatter into memory,
              renormalize touched slots.

Distribution: data-parallel over B across 8 NeuronCores; per-slot partial
update/denominator ReduceScattered over cores; each core renormalizes its
N-shard. Math notes:
  * softmax shift per update column is mathematically free; we use shift 0
    (values exp(max_sim) are in [1/e, e], no overflow), so no cross-core
    column-max pass is needed.
  * l2norm(mem + upd/denom) == l2norm(denom*mem + upd) for denom > 0, which
    lets the denominator division fold into the final renormalize.
"""
import sys

sys.path.insert(0, "/opt/trn_rl_repo")

import os

import numpy as np

import concourse.bass as bass
import concourse.mybir as mybir
import concourse.tile as tile
from concourse.bass_utils import run_bass_kernel_spmd
from concourse.masks import make_identity

B, N, H = 32768, 2048, 512
NCORES = 8
P = 128
BSH = B // NCORES            # 4096 rows per core
CH = BSH // P                # 32 chunks per core
NT = N // P                  # 16 N tiles
KH = H // P                  # 4 H (contraction) chunks
NSH = N // NCORES            # 256 slots per core
NTS = NSH // P               # 2 N-shard tiles

f32 = mybir.dt.float32
f32r = mybir.dt.float32r
bf16 = mybir.dt.bfloat16
AF = mybir.ActivationFunctionType
OP = mybir.AluOpType

# Similarity matmul dtype: f32r (TF32-like, 1 cyc/row) vs f32 (exact, 4 cyc/row).
PRECISE_SIM = False


def _split_multi_waits(nc):
    """This walrus build accepts one sync-wait slot per instruction; hoist
    extra waits emitted by Tile onto same-engine NOPs placed just before."""
    for bb in nc.main_func.blocks:
        new = []
        dirty = False
        for ins in bb.instructions:
            si = ins.sync_info
            if si is not None and len(si.on_wait) > 1:
                waits = list(si.on_wait)
                for k, w in enumerate(waits[:-1]):
                    nop = mybir.InstNoOp(name=f"{ins.name}.w{k}", ins=[], outs=[])
                    nop.engine = ins.engine
                    nop.sync_info = mybir.SyncInfo(on_wait=[w], on_update=[])
                    new.append(nop)
                ins.sync_info = mybir.SyncInfo(
                    on_wait=[waits[-1]], on_update=list(si.on_update)
                )
                new.append(ins)
                dirty = True
            else:
                new.append(ins)
        if dirty:
            bb.instructions = new


def build():
    nc = bass.Bass(target_bir_lowering=False)

    z_sh = nc.declare_dram_parameter("z_sh", [BSH, H], f32, isOutput=False)
    mem_in = nc.declare_dram_parameter("mem_in", [N, H], f32, isOutput=False)
    mem_shard = nc.declare_dram_parameter("mem_shard", [NSH, H], f32, isOutput=False)
    zhat_sh = nc.declare_dram_parameter("zhat_sh", [BSH, H], f32, isOutput=True)
    newmem_sh = nc.declare_dram_parameter("newmem_sh", [NSH, H], f32, isOutput=True)
    dbg_den = nc.declare_dram_parameter("dbg_den", [N], f32, isOutput=True)
    dbg_upd = nc.declare_dram_parameter("dbg_upd", [N, H], f32, isOutput=True)
    dbg_denrs = nc.declare_dram_parameter("dbg_denrs", [NSH], f32, isOutput=True)
    dbg_stash = nc.declare_dram_parameter("dbg_stash", [P, N], f32, isOutput=True)

    # internal DRAM
    stash = nc.dram_tensor("stash", [CH, P, N], bf16)         # u rows
    upd_loc = nc.dram_tensor("upd_loc", [N, H], f32)
    upd_rs = nc.dram_tensor("upd_rs", [NSH, H], f32)
    den_loc = nc.dram_tensor("den_loc", [N], f32)
    den_rs = nc.dram_tensor("den_rs", [NSH], f32)

    rg = [list(range(NCORES))]

    with tile.TileContext(nc) as tc:
        with (
            tc.tile_pool(name="const", bufs=1) as const,
            tc.tile_pool(name="resident", bufs=1) as res,
            tc.tile_pool(name="work", bufs=2) as work,
            tc.tile_pool(name="small", bufs=3) as small,
        ):
            from contextlib import ExitStack
            p1ctx = ExitStack()
            ps_sim = p1ctx.enter_context(tc.tile_pool(name="ps_sim", bufs=1, space="PSUM"))
            ps_tr = p1ctx.enter_context(tc.tile_pool(name="ps_tr", bufs=1, space="PSUM"))
            ps_etr = p1ctx.enter_context(tc.tile_pool(name="ps_etr", bufs=1, space="PSUM"))
            ps_zh = p1ctx.enter_context(tc.tile_pool(name="ps_zh", bufs=1, space="PSUM"))
            # ---------------- setup ----------------
            ident = const.tile([P, P], f32)
            make_identity(nc, ident[:])
            identb = const.tile([P, P], bf16)
            make_identity(nc, identb[:])
            ones_b = const.tile([P, 1], bf16)
            nc.vector.memset(ones_b[:], 1.0)

            # absorb gpsimd sem on PE so later transposes carry 1 wait
            dps = ps_tr.tile([P, P], f32, tag="tr")
            nc.tensor.transpose(dps[:], ident[:], ident[:])

            mem = res.tile([P, NT, H], f32)       # raw memory, tile nt = rows
            mem_b = res.tile([P, NT, H], bf16)    # bf16 copy for z_hat matmul
            nc.sync.dma_start(mem[:], mem_in.rearrange("(t p) h -> p t h", p=P))
            nc.vector.tensor_copy(mem_b[:], mem[:])

            # memory row norms -> normalized copy (f32r) -> transpose to mT
            msq = small.tile([P, NT], f32, tag="msq")
            for nt in range(NT):
                junk = work.tile([P, H], f32, tag="sqj")
                nc.scalar.activation(
                    junk[:], mem[:, nt], AF.Square, accum_out=msq[:, nt : nt + 1]
                )
            mnorm = small.tile([P, NT], f32, tag="mnorm")
            nc.scalar.activation(mnorm[:], msq[:], AF.Sqrt)
            mnc = small.tile([P, NT], f32, tag="mnc")
            nc.vector.tensor_scalar_max(mnc[:], mnorm[:], 1e-12)
            mrn = small.tile([P, NT], f32, tag="mrn")
            nc.vector.reciprocal(mrn[:], mnc[:])

            mT = res.tile([P, KH, N], f32r)       # m_n.T : [H-part, k, N]
            # raw mem f32 is only needed for mem_b/msq; normalize in place
            for nt in range(NT):
                nc.vector.tensor_scalar_mul(
                    mem[:, nt], mem[:, nt], mrn[:, nt : nt + 1]
                )
            for nt in range(NT):
                for k in range(KH):
                    tp = ps_tr.tile([P, P], f32, tag="tr")
                    nc.tensor.transpose(
                        tp[:], mem[:, nt, k * P : (k + 1) * P], ident[:]
                    )
                    if (nt + k) % 2 == 0:
                        nc.vector.tensor_copy(
                            mT[:, k, nt * P : (nt + 1) * P], tp[:]
                        )
                    else:
                        nc.scalar.activation(
                            mT[:, k, nt * P : (nt + 1) * P], tp[:], AF.Copy
                        )

            z_bf = res.tile([P, CH, H], bf16)     # bf16 z kept for pass 2

            # ---------------- prologue: all z row norms ----------------
            # (batched so the steady-state ACT runs Exp only, no table swaps)
            ssq_all = small.tile([P, CH], f32, tag="ssq_all")
            for c in range(CH):
                zp = work.tile([P, H], f32, tag="z_c")
                nc.sync.dma_start(zp[:], z_sh[c * P : (c + 1) * P, :])
                junk = work.tile([P, H], f32, tag="sqj")
                nc.scalar.activation(
                    junk[:], zp[:], AF.Square, accum_out=ssq_all[:, c : c + 1]
                )
            znorm_all = small.tile([P, CH], f32, tag="znorm_all")
            nc.scalar.activation(znorm_all[:], ssq_all[:], AF.Sqrt)
            znc_all = small.tile([P, CH], f32, tag="znc_all")
            nc.vector.tensor_scalar_max(znc_all[:], znorm_all[:], 1e-12)
            rnorm_all = res.tile([P, CH], f32)
            nc.vector.reciprocal(rnorm_all[:], znc_all[:])

            # ---------------- pass 1 ----------------
            for c in range(CH):
                rnorm = rnorm_all[:, c : c + 1]
                z_c = work.tile([P, H], f32, tag="z_c")
                nc.sync.dma_start(z_c[:], z_sh[c * P : (c + 1) * P, :])

                # z chunk transposed -> lhsT for similarity
                zT = work.tile([P, KH, P], f32r if not PRECISE_SIM else f32,
                               tag="zT")
                for k in range(KH):
                    tp = ps_tr.tile([P, P], f32, tag="tr")
                    nc.tensor.transpose(tp[:], z_c[:, k * P : (k + 1) * P], ident[:])
                    nc.vector.tensor_copy(zT[:, k], tp[:])

                # similarity: sim_raw = z . m_n  (rows of z unnormalized)
                sim = work.tile([P, N], f32, tag="sim")
                simps = [ps_sim.tile([P, 512], f32, tag=f"simp{n4}", name=f"simp{n4}")
                         for n4 in range(4)]
                for k in range(KH):
                    for n4 in range(4):
                        nc.tensor.matmul(
                            simps[n4][:],
                            zT[:, k],
                            mT[:, k, n4 * 512 : (n4 + 1) * 512],
                            start=(k == 0),
                            stop=(k == KH - 1),
                        )
                for n4 in range(4):
                    nc.vector.tensor_copy(sim[:, n4 * 512 : (n4 + 1) * 512],
                                          simps[n4][:])

                M = small.tile([P, 1], f32, tag="M")
                nc.vector.reduce_max(M[:], sim[:], axis=mybir.AxisListType.X)
                negMr = small.tile([P, 1], f32, tag="negMr")
                nc.vector.tensor_scalar(
                    negMr[:], M[:], rnorm, -1.0, OP.mult, OP.mult
                )
                expM = small.tile([P, 1], f32, tag="expM")
                nc.scalar.activation(expM[:], M[:], AF.Exp, scale=rnorm)

                expb = work.tile([P, N], bf16, tag="expb")
                sumexp = small.tile([P, 1], f32, tag="sumexp")
                nc.scalar.activation(
                    expb[:], sim[:], AF.Exp, bias=negMr[:], scale=rnorm,
                    accum_out=sumexp[:],
                )

                u_bf = work.tile([P, N], bf16, tag="u_bf")
                nc.vector.tensor_scalar(
                    u_bf[:], sim[:], M[:], expM[:], OP.is_ge, OP.mult
                )
                nc.sync.dma_start(stash[c], u_bf[:])

                # z_hat = (expb @ mem) * (1/sumexp)
                zhp = ps_zh.tile([P, H], f32, tag="zhp")
                eTp = ps_etr.tile([P, N], bf16, tag="etr")
                eT = work.tile([P, N], bf16, tag="eT")
                for half in range(2):
                    for j in range(8):
                        nt = half * 8 + j
                        nc.tensor.transpose(
                            eTp[:, nt * P : (nt + 1) * P],
                            expb[:, nt * P : (nt + 1) * P],
                            identb[:],
                        )
                    hs = slice(half * 1024, (half + 1) * 1024)
                    nc.scalar.activation(eT[:, hs], eTp[:, hs], AF.Copy)
                    for j in range(8):
                        nt = half * 8 + j
                        nc.tensor.matmul(
                            zhp[:], eT[:, nt * P : (nt + 1) * P], mem_b[:, nt],
                            start=(nt == 0), stop=(nt == NT - 1),
                        )
                rsum = small.tile([P, 1], f32, tag="rsum")
                nc.vector.reciprocal(rsum[:], sumexp[:])
                zh = work.tile([P, H], f32, tag="zh")
                nc.vector.tensor_scalar_mul(zh[:], zhp[:], rsum[:])
                nc.sync.dma_start(zhat_sh[c * P : (c + 1) * P, :], zh[:])

                nc.vector.tensor_copy(z_bf[:, c], z_c[:])

            p1ctx.close()
            # ---------------- pass 2: update + denom ----------------
            with (
                tc.tile_pool(name="ps_upd", bufs=1, space="PSUM") as ps_upd,
                tc.tile_pool(name="ps_den", bufs=1, space="PSUM") as ps_den,
            ):
                # den accumulates in SBUF: per-chunk psum tiles are written
                # with start=True each chunk, so the coarse has_written
                # granularity inside a shared bank cannot clobber running sums
                dsb = small.tile([P, NT], f32, tag="dsb")
                nc.vector.memset(dsb[:], 0.0)
                for nt0, nts in ((0, 7), (7, 7), (14, 2)):
                    updps = [ps_upd.tile([P, H], f32, tag=f"updp{j}", name=f"updp{j}")
                             for j in range(nts)]
                    for c in range(CH):
                        band = work.tile([P, nts * P], bf16, tag="band")
                        nc.sync.dma_start(
                            band[:, : nts * P],
                            stash[c][:, nt0 * P : (nt0 + nts) * P],
                        )
                        denc = ps_den.tile([P, nts], f32, tag="denc")
                        for j in range(nts):
                            u_t = band[:, j * P : (j + 1) * P]
                            nc.tensor.matmul(
                                updps[j][:], u_t, z_bf[:, c], start=(c == 0),
                                stop=(c == CH - 1),
                            )
                            nc.tensor.matmul(
                                denc[:, j : j + 1], u_t, ones_b[:],
                                start=True, stop=True,
                            )
                        nc.vector.tensor_add(
                            dsb[:, nt0 : nt0 + nts], dsb[:, nt0 : nt0 + nts],
                            denc[:],
                        )
                    for j in range(nts):
                        us = work.tile([P, H], f32, tag="us")
                        nc.scalar.activation(us[:], updps[j][:], AF.Copy)
                        nc.sync.dma_start(
                            upd_loc[(nt0 + j) * P : (nt0 + j + 1) * P, :], us[:]
                        )
                nc.sync.dma_start(
                    den_loc.rearrange("(t p) -> p t", p=P), dsb[:]
                )

            # ---------------- reduce-scatter + renormalize shard ----------
            nc.gpsimd.collective_compute(
                "ReduceScatter", OP.add, replica_groups=rg,
                ins=[upd_loc[:]], outs=[upd_rs[:]],
            )
            nc.gpsimd.collective_compute(
                "ReduceScatter", OP.add, replica_groups=rg,
                ins=[den_loc[:]], outs=[den_rs[:]],
            )

            # debug taps
            stcp = work.tile([P, N], bf16, tag="u_bf")
            nc.sync.dma_start(stcp[:], stash[5])
            stf = work.tile([P, N], f32, tag="sim")
            nc.vector.tensor_copy(stf[:], stcp[:])
            nc.sync.dma_start(dbg_stash[:], stf[:])
            dcp = work.tile([P, NT], f32, tag="dcp")
            nc.sync.dma_start(dcp[:], den_loc.rearrange("(t p) -> p t", p=P))
            nc.sync.dma_start(dbg_den.rearrange("(t p) -> p t", p=P), dcp[:])
            for nt in range(NT):
                ucp = work.tile([P, H], f32, tag="ucp")
                nc.sync.dma_start(ucp[:], upd_loc[nt * P : (nt + 1) * P, :])
                nc.sync.dma_start(dbg_upd[nt * P : (nt + 1) * P, :], ucp[:])
            dcp2 = work.tile([P, NTS], f32, tag="dcp2")
            nc.sync.dma_start(dcp2[:], den_rs.rearrange("(t p) -> p t", p=P))
            nc.sync.dma_start(dbg_denrs.rearrange("(t p) -> p t", p=P), dcp2[:])

            dsh = small.tile([P, NTS], f32, tag="dsh")
            nc.sync.dma_start(dsh[:], den_rs.rearrange("(t p) -> p t", p=P))
            for t in range(NTS):
                mt = work.tile([P, H], f32, tag="mt")
                nc.sync.dma_start(mt[:], mem_shard[t * P : (t + 1) * P, :])
                ut = work.tile([P, H], f32, tag="ut")
                nc.sync.dma_start(ut[:], upd_rs[t * P : (t + 1) * P, :])

                dcol = dsh[:, t : t + 1]
                sel = small.tile([P, 1], f32, tag="sel")
                nc.vector.tensor_scalar(sel[:], dcol, 0.5, None, OP.is_ge)
                inv = small.tile([P, 1], f32, tag="inv")  # 1 - sel
                nc.vector.tensor_scalar(inv[:], sel[:], -1.0, 1.0, OP.mult, OP.add)

                nm = work.tile([P, H], f32, tag="nm")
                nc.vector.tensor_scalar_mul(nm[:], mt[:], dcol)     # denom*mem
                nc.vector.tensor_add(nm[:], nm[:], ut[:])           # + upd
                keep = work.tile([P, H], f32, tag="keep")
                nc.vector.tensor_scalar_mul(keep[:], mt[:], inv[:])
                nc.vector.tensor_add(nm[:], nm[:], keep[:])

                sq = small.tile([P, 1], f32, tag="sq2")
                junk2 = work.tile([P, H], f32, tag="sqj")
                nc.scalar.activation(junk2[:], nm[:], AF.Square, accum_out=sq[:])
                nrm = small.tile([P, 1], f32, tag="nrm")
                nc.scalar.activation(nrm[:], sq[:], AF.Sqrt)
                nrmc = small.tile([P, 1], f32, tag="nrmc")
                nc.vector.tensor_scalar_max(nrmc[:], nrm[:], 1e-12)
                # divisor = sel*norm + (1-sel)
                dv = small.tile([P, 1], f32, tag="dv")
                nc.vector.tensor_scalar_mul(dv[:], nrmc[:], sel[:])
                nc.vector.tensor_add(dv[:], dv[:], inv[:])
                rdv = small.tile([P, 1], f32, tag="rdv")
                nc.vector.reciprocal(rdv[:], dv[:])
                out_t = work.tile([P, H], f32, tag="out_t")
                nc.vector.tensor_scalar_mul(out_t[:], nm[:], rdv[:])
                nc.sync.dma_start(newmem_sh[t * P : (t + 1) * P, :], out_t[:])

    _split_multi_waits(nc)
    return nc


_NC_CACHE = None
LAST_RESULT = None


def kernel(z: np.ndarray, memory_items: np.ndarray):
    global _NC_CACHE
    if _NC_CACHE is None:
        _NC_CACHE = build()
    nc = _NC_CACHE

    z = np.ascontiguousarray(z, dtype=np.float32)
    mem = np.ascontiguousarray(memory_items, dtype=np.float32)
    in_maps = []
    for c in range(NCORES):
        in_maps.append(
            {
                "z_sh": z[c * BSH : (c + 1) * BSH],
                "mem_in": mem,
                "mem_shard": mem[c * NSH : (c + 1) * NSH],
            }
        )
    trace = bool(os.environ.get("KER_TRACE"))
    res = run_bass_kernel_spmd(
        nc, in_maps, list(range(NCORES)), trace=trace,
        trace_cores=[0] if trace else None,
    )
    global LAST_RESULT
    LAST_RESULT = res
    z_hat = np.concatenate([res.results[c]["zhat_sh"] for c in range(NCORES)], axis=0)
    new_mem = np.concatenate(
        [res.results[c]["newmem_sh"] for c in range(NCORES)], axis=0
    )
    return z_hat, new_mem


if __name__ == "__main__":
    rng = np.random.default_rng(0)
    z = rng.standard_normal((B, H)).astype(np.float32)
    m = rng.standard_normal((N, H)).astype(np.float32)
    zh, nm = kernel(z=z, memory_items=m)
    print(zh.shape, nm.shape, np.abs(zh).max(), np.abs(nm).max())
